# revision 3
# baseline (speedup 1.0000x reference)
"""AsyncCrossModalConsistencyLoss distributed Bass kernel for 8 TRN2 NeuronCores.

Data-parallel: batch dim (B=8) sharded one element per core. Each core:
  - casts its [4096, 512] visual/audio shard f32->bf16 during the DMA
  - per [128,512] tile: row sum-of-squares (ScalarE activation accum /
    VectorE tensor_tensor_reduce, alternating for balance), row dots
    (VectorE ttr), 1/norm (Sqrt + reciprocal in f32), then TensorE matmuls
    accumulate sum_s v_hat (bf16, full rate) and the sync dot-sum in PSUM
  - epilogue computes the margin loss, pre-scaled by 1/8
AllReduce(add) over the 8 cores produces the global mean loss.
"""

import numpy as np

import concourse.bass as bass
import concourse.tile as tile
from concourse import bacc, mybir
from concourse.bass_utils import run_bass_kernel_spmd

N_CORES = 8
S = 4096
D = 512
P = 128
NT = S // P              # 32 compute tiles of [128, 512]
FREE = NT * D            # 16384 columns per partition
TILES_PER_CHUNK = 4          # DMA chunk (1 MB per tensor per chunk)
NCH = NT // TILES_PER_CHUNK
CHUNK_COLS = TILES_PER_CHUNK * D
CTPC = 4                     # compute chunk: tiles per batched-norm group
CNCH = NT // CTPC

EPS_DIV = 1e-8
MARGIN = 0.5
C_SYNC = 1.0 / S
C_ASYNC = 1.0 / (S * (S - 1) + EPS_DIV)

F32 = mybir.dt.float32
BF16 = mybir.dt.bfloat16
AF = mybir.ActivationFunctionType
OP = mybir.AluOpType


def _build(collective=True, reps=1):
    """reps>1: wrap the body in tc.For_i for differential timing (no
    collective in that mode — collectives can't sit in control flow)."""
    import contextlib

    nc = bacc.Bacc(
        "TRN2", target_bir_lowering=False, debug=False,
        num_devices=N_CORES if collective else 1,
    )
    v_ext = nc.dram_tensor("v", [S, D], F32, kind="ExternalInput")
    a_ext = nc.dram_tensor("a", [S, D], F32, kind="ExternalInput")
    w_ext = nc.dram_tensor("w", [1, 1], F32, kind="ExternalInput")
    out_ext = nc.dram_tensor("out", [1, 1], F32, kind="ExternalOutput")

    # Row s = p*NT + n lands on partition p, tile n: contiguous 64KB per
    # partition in DRAM -> ideal DMA pattern. Any row->(p,n) bijection works
    # because every reduction here is symmetric over rows.
    v_re = v_ext.ap().rearrange("(p n) d -> p (n d)", p=P)
    a_re = a_ext.ap().rearrange("(p n) d -> p (n d)", p=P)

    with tile.TileContext(nc) as tc:
        with (
            tc.tile_pool(name="big", bufs=1) as big,
            tc.tile_pool(name="scratch", bufs=3) as scratch,
            tc.tile_pool(name="small", bufs=6) as small,
            tc.tile_pool(name="psum", bufs=1, space="PSUM") as psum,
            tc.tile_pool(name="dram", bufs=1, space="DRAM") as dram,
        ):
            v_sb = big.tile([P, FREE], BF16)
            a_sb = big.tile([P, FREE], BF16)
            w_sb = big.tile([1, 1], F32)
            eps_b = big.tile([P, 1], F32)
            nc.vector.memset(eps_b[:], 1e-24)
            nc.sync.dma_start(w_sb[:], w_ext[:])
            loop_cm = tc.For_i(0, reps) if reps > 1 else contextlib.nullcontext()
            with loop_cm:
                _body(nc, tc, scratch, small, psum, v_sb, a_sb, w_sb, eps_b,
                      v_re, a_re)
            lscaled = _EPILOGUE_OUT[0]

            if collective:
                loss_bounce = dram.tile([1, 1], F32)
                out_bounce = dram.tile([1, 1], F32)
                nc.gpsimd.dma_start(loss_bounce[:], lscaled[:])
                nc.gpsimd.collective_compute(
                    "AllReduce",
                    OP.add,
                    replica_groups=[list(range(N_CORES))],
                    ins=[loss_bounce.opt()],
                    outs=[out_bounce.opt()],
                )
                nc.gpsimd.dma_start(out_ext[:], out_bounce[:])
            else:
                nc.sync.dma_start(out_ext[:], lscaled[:])

    nc.compile()
    return nc


_EPILOGUE_OUT = [None]


def _body(nc, tc, scratch, small, psum, v_sb, a_sb, w_sb, eps_b, v_re, a_re):
            # Uniform 1 MB DMA chunks. Measured best: tapering the edge
            # chunks (2-tile) to shorten fill/tail costs more in extra SWDGE
            # descgen + per-DMA completion latency than it saves (+6 us/iter).
            for c in range(NCH):
                sl = slice(c * CHUNK_COLS, (c + 1) * CHUNK_COLS)
                # gpsimd (SWDGE) DMA casts f32 -> bf16 in flight
                nc.gpsimd.dma_start(v_sb[:, sl], v_re[:, sl])
                nc.gpsimd.dma_start(a_sb[:, sl], a_re[:, sl])

            sumv_ps = psum.tile([1, D], F32)
            suma_ps = psum.tile([1, D], F32)
            sync_ps = psum.tile([1, D], F32)

            TPC = CTPC
            for c in range(CNCH):
                first = c == 0
                last = c == CNCH - 1
                # ss: cols [0:TPC] = sum v^2 per tile, [TPC:2TPC] = sum a^2
                ss = small.tile([P, 2 * TPC], F32)
                prods = []
                for j in range(TPC):
                    t = c * TPC + j
                    sl = slice(t * D, (t + 1) * D)
                    v_t = v_sb[:, sl]
                    a_t = a_sb[:, sl]
                    sq_v = scratch.tile([P, D], BF16)
                    nc.scalar.activation(
                        sq_v[:], v_t, AF.Square, accum_out=ss[:, j:j + 1]
                    )
                    sq_a = scratch.tile([P, D], BF16)
                    if j < 1:
                        # ScalarE takes 5 of the 8 square-reduce passes per
                        # chunk (4 sq_v + this one), DVE the other 3 via
                        # scalar_tensor_tensor accum (InstTensorTensorReduce
                        # faults on this HW)
                        nc.scalar.activation(
                            sq_a[:], a_t, AF.Square,
                            accum_out=ss[:, TPC + j:TPC + j + 1],
                        )
                    else:
                        nc.vector.scalar_tensor_tensor(
                            out=sq_a[:], in0=a_t, scalar=1.0, in1=a_t,
                            op0=OP.mult, op1=OP.mult,
                            accum_out=ss[:, TPC + j:TPC + j + 1],
                        )
                    # prod = v*a (bf16 2x mode); its weighted row-sum goes
                    # through the PE below, so no per-row dot accum is needed
                    prod = scratch.tile([P, D], BF16, tag=f"prod{j}")
                    nc.vector.tensor_tensor(
                        out=prod[:], in0=v_t, in1=a_t, op=OP.mult
                    )
                    prods.append(prod)

                # Batched 1/max(norm, eps) for the whole chunk. The sqrt bias
                # keeps sqrt(0) finite, matching F.normalize's max(norm, 1e-12)
                # for all realizable inputs.
                nrm = small.tile([P, 2 * TPC], F32)
                nc.scalar.activation(nrm[:], ss[:], AF.Sqrt, bias=eps_b[:])
                inv = small.tile([P, 2 * TPC], F32)
                nc.vector.reciprocal(inv[:], nrm[:])
                inv_b = small.tile([P, 2 * TPC], BF16)
                nc.vector.tensor_copy(inv_b[:], inv[:])
                invva_b = small.tile([P, TPC], BF16)
                nc.vector.tensor_mul(invva_b[:], inv[:, 0:TPC], inv[:, TPC:])

                for j in range(TPC):
                    t = c * TPC + j
                    sl = slice(t * D, (t + 1) * D)
                    st = first and j == 0
                    sp = last and j == TPC - 1
                    nc.tensor.matmul(
                        sumv_ps[:], lhsT=inv_b[:, j:j + 1], rhs=v_sb[:, sl],
                        start=st, stop=sp,
                    )
                    nc.tensor.matmul(
                        suma_ps[:], lhsT=inv_b[:, TPC + j:TPC + j + 1],
                        rhs=a_sb[:, sl],
                        start=st, stop=sp,
                    )
                    # sync row: [1,D] += invva.T @ (v*a); summed in epilogue
                    nc.tensor.matmul(
                        sync_ps[:], lhsT=invva_b[:, j:j + 1], rhs=prods[j][:],
                        start=st, stop=sp,
                    )

            # ---- epilogue: scalars on partition 0 ----
            suma_sb = small.tile([1, D], F32)
            nc.scalar.copy(suma_sb[:], suma_ps[:])
            prod_e = scratch.tile([P, D], F32)
            total = small.tile([1, 1], F32)
            nc.vector.scalar_tensor_tensor(
                out=prod_e[0:1, :], in0=sumv_ps[:], scalar=1.0, in1=suma_sb[:],
                op0=OP.mult, op1=OP.mult, accum_out=total[:],
            )
            sync_sb = small.tile([1, 1], F32)
            nc.vector.tensor_reduce(
                out=sync_sb[:], in_=sync_ps[:], op=OP.add,
                axis=mybir.AxisListType.X,
            )
            # diff = async_mean - sync_mean = total*C_ASYNC - sync*(C_SYNC+C_ASYNC)
            tmp = small.tile([1, 1], F32)
            nc.vector.tensor_scalar_mul(tmp[:], sync_sb[:], C_SYNC + C_ASYNC)
            diff = small.tile([1, 1], F32)
            nc.vector.scalar_tensor_tensor(
                out=diff[:], in0=total[:], scalar=C_ASYNC, in1=tmp[:],
                op0=OP.mult, op1=OP.subtract,
            )
            marg = small.tile([1, 2], F32)
            nc.vector.tensor_scalar_add(marg[:, 0:1], diff[:], MARGIN)
            nc.vector.tensor_scalar(
                marg[:, 1:2], diff[:], -1.0, MARGIN * 0.1, op0=OP.mult, op1=OP.add
            )
            relu = small.tile([1, 2], F32)
            nc.vector.tensor_scalar_max(relu[:], marg[:], 0.0)
            # loss/8 = (relu1 + w*(relu0 - relu1)) / 8
            d01 = small.tile([1, 1], F32)
            nc.vector.tensor_sub(d01[:], relu[:, 0:1], relu[:, 1:2])
            wd = small.tile([1, 1], F32)
            nc.vector.tensor_mul(wd[:], d01[:], w_sb[:])
            lsum = small.tile([1, 1], F32)
            nc.vector.tensor_add(lsum[:], wd[:], relu[:, 1:2])
            lscaled = small.tile([1, 1], F32)
            nc.vector.tensor_scalar_mul(lscaled[:], lsum[:], 1.0 / N_CORES)
            _EPILOGUE_OUT[0] = lscaled


_NC = None


def _get_nc():
    global _NC
    if _NC is None:
        _NC = _build()
    return _NC


def make_in_maps(visual_features, audio_features, targets):
    vf = np.asarray(visual_features)
    af = np.asarray(audio_features)
    tg = np.asarray(targets)
    return [
        {
            "v": np.ascontiguousarray(vf[i], dtype=np.float32),
            "a": np.ascontiguousarray(af[i], dtype=np.float32),
            "w": np.array([[float(tg[i])]], dtype=np.float32),
        }
        for i in range(N_CORES)
    ]


def kernel(visual_features, audio_features, targets):
    nc = _get_nc()
    in_maps = make_in_maps(visual_features, audio_features, targets)
    res = run_bass_kernel_spmd(nc, in_maps, core_ids=list(range(N_CORES)))
    out = np.asarray(res.results[0]["out"], dtype=np.float32)
    return out.reshape(())


if __name__ == "__main__":
    rng = np.random.default_rng(0)
    v = rng.standard_normal((N_CORES, S, D)).astype(np.float32)
    a = rng.standard_normal((N_CORES, S, D)).astype(np.float32)
    t = rng.integers(0, 2, (N_CORES,)).astype(np.int32)
    print(kernel(visual_features=v, audio_features=a, targets=t))

